# revision 43
# baseline (speedup 1.0000x reference)
"""MoE layer (nn_MoELayer_4681514353281) Trainium2 Bass kernel.

Reference semantics: for slot i in range(4), expert i's FFN (W1 + A1@B1 LoRA,
gelu-tanh, W2 + A2@B2 LoRA) runs densely over ALL tokens; per-token combine
weight = renormalized top-4 softmax gate weight where top_idx == i (else 0).
Only experts 0-3 are ever used.

Token gathering: a token contributes to expert i only when i is in its top-4
(~1/4 of tokens per expert), so each core processes just the gathered
contributing tokens (~2.1k instead of all 8192) — an exact 4x FLOP cut, since
dropped (token, expert) pairs have combine weight exactly 0.

Sharding: 8 cores x 2 segments = 16 work units (expert, F-quarter). Each core
gets one unit from the 8 largest and one from the 8 smallest (pairing), so
expert token-count imbalance doesn't pad every core to the largest expert.
A core's weight input [D, 2048] holds the two quarter-slices of W1c/W2c
(W1c = W1 + A1@B1, W2c = W2 + A2@B2 — LoRA folded on host, exact identity);
its x input concatenates the two gathered token streams. Segment A uses
weight columns fc 0-7, segment B fc 8-15. Host scatter-adds the 16 partial
outputs (4 F-quarter partials per (token, expert) pair).

The gate's top-4 selection needs ~1e-6 logit precision to reproduce the fp32
reference's picks (near-ties flip otherwise), so the 8192x16 softmax/top-4
(0.13% of FLOPs) is computed on the host. FFN operands are bf16 (same
78.6 TF/s PE rate as float32r, half the DMA/SBUF, no small-moving-dim
penalty); accumulation is fp32 in PSUM. bf16 adds ~3e-3 relative error,
well inside the 2e-2 gate.

Schedule notes (TimelineSim-tuned):
- DMA loads are need-ordered: x block 0, w1 fc0-7, wc, w2 fc0-7, then the
  segment-B halves. HWDGE costs ~630ns per copy (128 descriptors) so
  startup-critical copies must not queue behind later ones.
- 10 warmup matmuls on a zeroed tile keep the PE busy until the first real
  operands land: an idle gap resets the PE p-state ramp (0.65->2.4GHz
  after 3us continuous), which would halve early matmul throughput.
- Output DMAs ride the sync queue so they never contend with x prefetch.
- Block sizes <= 512 (PSUM bank), descending (512, ..., 384, 256): big
  early blocks hide the weight stream; the last down accumulator is split
  in halves to trim the end-of-kernel drain.
"""

import os
import sys

sys.path.insert(0, "/opt/trn_rl_repo")

import ml_dtypes
import numpy as np

# Problem dims (hardcoded per spec)
B, S, D, F, E, R = 2, 4096, 1024, 4096, 16, 16
TOPK = 4
N_TOK = B * S          # 8192
F2 = F // 2            # 2048 weight columns per core
DC = D // 128          # 8
FC = F2 // 128         # 16
WARM_N = 10            # PE p-state warmup matmuls
XB0_SPLIT = 1          # way-split of the first x block load
HAP_BUFS = FC + 2      # h tile pool depth
TAIL_SPLIT = True      # split last down accumulator to trim end drain
TAIL_OUTQ = False      # (sim: scalar-queue tail outs were slightly worse)

_programs = {}
LAST_RESULTS = None
LAST_PROGRAM = None


def _build_program(segments):
    """segments: tuple of (blocks, fc_lo, fc_hi, up_len). Each segment
    processes sum(blocks) gathered tokens against the fc range
    [fc_lo, fc_hi) of the weight tensors (the expert/F-quarter pairing
    described above); its up-projection streams only up_len columns."""
    import concourse.tile as tile
    from concourse import bacc, mybir

    BF16 = mybir.dt.bfloat16
    F32 = mybir.dt.float32
    AF = mybir.ActivationFunctionType

    n_pad = sum(sum(blocks) for blocks, _, _, _ in segments)
    ncol = n_pad // 128

    nc = bacc.Bacc("TRN2", target_bir_lowering=False, debug=False, num_devices=8)

    xTd = nc.dram_tensor("xT", [D, n_pad], BF16, kind="ExternalInput")
    w1d = nc.dram_tensor("w1", [D, F2], BF16, kind="ExternalInput")
    w2d = nc.dram_tensor("w2", [F2, D], BF16, kind="ExternalInput")
    wcd = nc.dram_tensor("wc", [128, ncol], F32, kind="ExternalInput")
    outd = nc.dram_tensor("out", [n_pad, D], BF16, kind="ExternalOutput")

    with tile.TileContext(nc) as tc:
        with (
            tc.tile_pool(name="singles", bufs=1) as singles,
            tc.tile_pool(name="xp", bufs=2) as xp,
            tc.tile_pool(name="hap", bufs=HAP_BUFS) as hap,
            tc.tile_pool(name="outp", bufs=3) as outp,
            tc.tile_pool(name="psH", bufs=3, space="PSUM") as psH,
            tc.tile_pool(name="psEO", bufs=5, space="PSUM") as psEO,
        ):
            # ---- resident weights ----
            w1 = singles.tile([128, FC, DC, 128], BF16)   # [p, fc, dc, q]
            w2 = singles.tile([128, FC, D], BF16)         # [p, fc, d]
            w_all = singles.tile([128, ncol], F32)

            xT_r = xTd.rearrange("(dc p) t -> p dc t", p=128)
            w1_r = w1d.rearrange("(dc p) (fc q) -> p fc dc q", p=128, q=128)
            w2_r = w2d.rearrange("(fc p) d -> p fc d", p=128)

            def load_block(t0, bs, nsplit=1):
                t = xp.tile([128, DC, bs], BF16, tag="xb")
                step = DC // nsplit
                for d0 in range(0, DC, step):
                    nc.scalar.dma_start(
                        t[:, d0:d0 + step, :], xT_r[:, d0:d0 + step, t0:t0 + bs]
                    )
                return t

            # flatten segments into a linear block schedule; each entry
            # carries bs_up <= bs: the exact token count the up-proj must
            # stream (down-proj stays 128-aligned; surplus h columns land
            # in zero-weight rows the host never reads)
            sched = []
            for blocks, fc_lo, fc_hi, up_len in segments:
                off = 0
                for bs in blocks:
                    bs_up = max(0, min(bs, up_len - off))
                    sched.append((bs, fc_lo, fc_hi, bs_up))
                    off += bs

            # need-ordered loads: segment A reads w1 fc0-7 from ~4us and
            # w2 fc0-7 from ~30us; the fc8-15 halves only at segment B
            xb = load_block(0, sched[0][0], nsplit=XB0_SPLIT)  # bs of block 0
            half = FC // 2
            for fc in range(half):
                nc.sync.dma_start(w1[:, fc, :, :], w1_r[:, fc, :, :])
            nc.sync.dma_start(w_all[:], wcd[:, :])
            for fc in range(half):
                nc.sync.dma_start(w2[:, fc, :], w2_r[:, fc, :])
            for fc in range(half, FC):
                nc.sync.dma_start(w1[:, fc, :, :], w1_r[:, fc, :, :])
            for fc in range(half, FC):
                nc.sync.dma_start(w2[:, fc, :], w2_r[:, fc, :])

            # PE p-state warmup (see module docstring)
            if WARM_N:
                warm = singles.tile([128, 512], BF16)
                nc.gpsimd.memset(warm[:], 0.0)
                ps_w = psEO.tile([128, 512], F32, tag="eo")
                for i in range(WARM_N):
                    nc.tensor.matmul(
                        ps_w[:], warm[:, :128], warm[:],
                        start=(i == 0), stop=(i == WARM_N - 1),
                    )

            t0 = 0
            for blk, (bs, fc_lo, fc_hi, bs_up) in enumerate(sched):
                # up projection: h[fc][:, t] = gelu(x @ W1c)[f, t]
                h_all = {}
                for fc in range(fc_lo, fc_hi):
                    ps_h = psH.tile([128, bs_up], F32, tag="psh")
                    for dc in range(DC):
                        nc.tensor.matmul(
                            ps_h[:], w1[:, fc, dc, :], xb[:, dc, :bs_up],
                            start=(dc == 0), stop=(dc == DC - 1),
                        )
                    h = hap.tile([128, bs], BF16, tag="h")
                    nc.scalar.activation(h[:, :bs_up], ps_h[:], AF.Gelu_apprx_tanh)
                    h_all[fc] = h

                # prefetch next block's x while the down passes run
                if blk + 1 < len(sched):
                    xb_next = load_block(t0 + bs, sched[blk + 1][0])
                else:
                    xb_next = None

                # down projection in two d-half passes, 128-token columns
                last_blk = blk == len(sched) - 1
                for dh in range(2):
                    for sub in range(bs // 128):
                        col = t0 // 128 + sub
                        r0 = t0 + sub * 128
                        # final accumulator split in halves: the first
                        # half's combine+DMA overlaps the second's matmuls,
                        # trimming the end-of-kernel drain
                        final = (TAIL_SPLIT and last_blk and dh == 1
                                 and sub == bs // 128 - 1)
                        for piece in ([(0, 256), (256, 512)] if final
                                      else [(0, 512)]):
                            p0, p1 = piece
                            pw = p1 - p0
                            eo = psEO.tile([128, pw], F32, tag="eo")
                            for fc in range(fc_lo, fc_hi):
                                nc.tensor.matmul(
                                    eo[:],
                                    h_all[fc][:, sub * 128:(sub + 1) * 128],
                                    w2[:, fc, dh * 512 + p0:dh * 512 + p1],
                                    start=(fc == fc_lo), stop=(fc == fc_hi - 1),
                                )
                            ob = outp.tile([128, pw], BF16, tag="ob")
                            nc.vector.tensor_scalar_mul(
                                ob[:], eo[:], scalar1=w_all[:, col:col + 1]
                            )
                            # last block's outputs ride the then-idle
                            # scalar queue (no xb prefetch left) so the
                            # final copy isn't stuck behind queued outs
                            oq = nc.scalar if (TAIL_OUTQ and last_blk) else nc.sync
                            oq.dma_start(
                                outd[r0:r0 + 128,
                                     dh * 512 + p0:dh * 512 + p1], ob[:]
                            )

                xb = xb_next
                t0 += bs

    nc.compile()
    return nc


def _get_program(segments):
    segments = tuple(segments)
    if segments not in _programs:
        _programs[segments] = _build_program(segments)
    return _programs[segments]


def _block_split(n_pad):
    """Split n_pad (multiple of 128) into blocks of <= 512 (PSUM bank),
    descending: big early blocks hide the weight-stream DMA (sim-verified
    faster than ascending; non-monotone orders broke PJRT execution)."""
    if n_pad <= 512:
        return (n_pad,)
    q, r = divmod(n_pad, 512)
    if r == 0:
        return (512,) * q
    if r == 128:
        # 384+256 instead of a 128 tail (keeps blocks >= 256)
        return (512,) * (q - 1) + (384, 256)
    return (512,) * q + (r,)


def _gate_weights(x2d, Wg):
    """Reference-faithful gate (same ops as the reference, jax on CPU so the
    fp32 softmax/top-4 selection matches bit-for-bit). Returns [N_TOK, 4]
    combine weights for experts 0-3."""
    try:
        import jax
        import jax.numpy as jnp
        cpu = jax.devices("cpu")[0]
        with jax.default_device(cpu):
            xf = jnp.asarray(x2d, jnp.float32)
            wg = jnp.asarray(Wg, jnp.float32)
            weights = jax.nn.softmax(xf @ wg, axis=-1)
            top_w, top_idx = jax.lax.top_k(weights, TOPK)
            top_w = top_w / jnp.sum(top_w, axis=-1, keepdims=True)
            cols = [jnp.sum(top_w * (top_idx == i), axis=-1) for i in range(TOPK)]
            return np.asarray(jnp.stack(cols, axis=-1), np.float32)
    except Exception:
        # numpy fallback (identical math, BLAS rounding may differ ~1e-7)
        logits = x2d.astype(np.float32) @ Wg.astype(np.float32)
        m = logits.max(axis=-1, keepdims=True)
        e = np.exp((logits - m).astype(np.float32), dtype=np.float32)
        p = (e / e.sum(axis=-1, keepdims=True).astype(np.float32)).astype(np.float32)
        idx = np.argsort(-p, axis=-1, kind="stable")[:, :TOPK]
        topw = np.take_along_axis(p, idx, axis=-1)
        topw = (topw / topw.sum(axis=-1, keepdims=True)).astype(np.float32)
        w = np.zeros((x2d.shape[0], TOPK), np.float32)
        for i in range(TOPK):
            w[:, i] = (topw * (idx == i)).sum(axis=-1)
        return w


def kernel(x, Wg, W1, A1, B1, W2, A2, B2):
    global LAST_RESULTS, LAST_PROGRAM
    from concourse.bass_utils import run_bass_kernel_spmd

    x = np.asarray(x, dtype=np.float32)
    x2d = x.reshape(N_TOK, D)
    w4 = _gate_weights(x2d, np.asarray(Wg, dtype=np.float32))

    # gather contributing tokens per expert (combine weight exactly 0 else)
    idxs = [np.nonzero(w4[:, e])[0] for e in range(TOPK)]
    counts = [len(ix) for ix in idxs]
    pads = [max(128, -(-c // 128) * 128) for c in counts]

    # 16 work units (expert, F-quarter), each sized pads[e]. Pair the 8
    # largest with the 8 smallest so every core gets an equal token budget
    # (expert imbalance otherwise pads every core to the largest expert).
    units = sorted(
        ((pads[e], e, q) for e in range(TOPK) for q in range(4)), reverse=True
    )
    big, small = units[:8], units[8:]
    nA, nB = big[0][0], small[0][0]
    upA = max(counts[e] for _, e, _ in big)
    upB = max(counts[e] for _, e, _ in small)
    segments = ((_block_split(nA), 0, FC // 2, upA),
                (_block_split(nB), FC // 2, FC, upB))
    n_pad = nA + nB
    ncol = n_pad // 128
    FQ = F // 4  # 1024 weight columns per quarter

    nc = _get_program(segments)
    LAST_PROGRAM = nc

    bf16 = ml_dtypes.bfloat16
    x2dT_b = x2d.T.astype(bf16)  # [D, N] in bf16
    folded = []
    for e in range(TOPK):
        # fold the rank-16 LoRA into the dense weights (exact identity)
        w1c = (np.asarray(W1[e], np.float64)
               + np.asarray(A1[e], np.float64) @ np.asarray(B1[e], np.float64))
        w2c = (np.asarray(W2[e], np.float64)
               + np.asarray(A2[e], np.float64) @ np.asarray(B2[e], np.float64))
        folded.append((w1c.astype(bf16), w2c.astype(bf16)))

    in_maps = []
    placements = []  # per core: ((eA, cA), (eB, cB)) for output assembly
    for core in range(8):
        (szA, eA, qA), (szB, eB, qB) = big[core], small[core]
        xg = np.zeros((D, n_pad), bf16)
        xg[:, :counts[eA]] = x2dT_b[:, idxs[eA]]
        xg[:, nA:nA + counts[eB]] = x2dT_b[:, idxs[eB]]
        wg = np.zeros(n_pad, np.float32)
        wg[:counts[eA]] = w4[idxs[eA], eA]
        wg[nA:nA + counts[eB]] = w4[idxs[eB], eB]
        wc = np.ascontiguousarray(wg.reshape(ncol, 128).T)
        w1A, w2A = folded[eA]
        w1B, w2B = folded[eB]
        w1 = np.hstack([w1A[:, qA * FQ:(qA + 1) * FQ],
                        w1B[:, qB * FQ:(qB + 1) * FQ]])
        w2 = np.vstack([w2A[qA * FQ:(qA + 1) * FQ, :],
                        w2B[qB * FQ:(qB + 1) * FQ, :]])
        in_maps.append({
            "xT": xg,
            "w1": np.ascontiguousarray(w1),
            "w2": np.ascontiguousarray(w2),
            "wc": wc,
        })
        placements.append(((eA, counts[eA]), (eB, counts[eB])))

    trace = bool(os.environ.get("KERNEL_TRACE"))
    res = None
    last_exc = None
    for _attempt in range(3):
        try:
            res = run_bass_kernel_spmd(
                nc, in_maps, core_ids=list(range(8)), trace=trace
            )
            break
        except Exception as exc:  # transient NRT/profiling faults — retry
            last_exc = exc
            trace = False
    if res is None:
        raise last_exc
    LAST_RESULTS = res

    out = np.zeros((N_TOK, D), np.float64)
    for core in range(8):
        o = res.results[core]["out"]
        (eA, cA), (eB, cB) = placements[core]
        out[idxs[eA]] += o[:cA].astype(np.float64)
        out[idxs[eB]] += o[nA:nA + cB].astype(np.float64)
    return out.astype(np.float32).reshape(B, S, D)


# revision 44
# speedup vs baseline: 1.0106x; 1.0106x over previous
"""MoE layer (nn_MoELayer_4681514353281) Trainium2 Bass kernel.

Reference semantics: for slot i in range(4), expert i's FFN (W1 + A1@B1 LoRA,
gelu-tanh, W2 + A2@B2 LoRA) runs densely over ALL tokens; per-token combine
weight = renormalized top-4 softmax gate weight where top_idx == i (else 0).
Only experts 0-3 are ever used.

Token gathering: a token contributes to expert i only when i is in its top-4
(~1/4 of tokens per expert), so each core processes just the gathered
contributing tokens (~2.1k instead of all 8192) — an exact 4x FLOP cut, since
dropped (token, expert) pairs have combine weight exactly 0.

Sharding: 8 cores x 2 segments = 16 work units (expert, F-quarter). Each core
gets one unit from the 8 largest and one from the 8 smallest (pairing), so
expert token-count imbalance doesn't pad every core to the largest expert.
A core's weight input [D, 2048] holds the two quarter-slices of W1c/W2c
(W1c = W1 + A1@B1, W2c = W2 + A2@B2 — LoRA folded on host, exact identity);
its x input concatenates the two gathered token streams. Segment A uses
weight columns fc 0-7, segment B fc 8-15. Host scatter-adds the 16 partial
outputs (4 F-quarter partials per (token, expert) pair).

The gate's top-4 selection needs ~1e-6 logit precision to reproduce the fp32
reference's picks (near-ties flip otherwise), so the 8192x16 softmax/top-4
(0.13% of FLOPs) is computed on the host. FFN operands are bf16 (same
78.6 TF/s PE rate as float32r, half the DMA/SBUF, no small-moving-dim
penalty); accumulation is fp32 in PSUM. bf16 adds ~3e-3 relative error,
well inside the 2e-2 gate.

Schedule notes (TimelineSim-tuned):
- DMA loads are need-ordered: x block 0, w1 fc0-7, wc, w2 fc0-7, then the
  segment-B halves. HWDGE costs ~630ns per copy (128 descriptors) so
  startup-critical copies must not queue behind later ones.
- 10 warmup matmuls on a zeroed tile keep the PE busy until the first real
  operands land: an idle gap resets the PE p-state ramp (0.65->2.4GHz
  after 3us continuous), which would halve early matmul throughput.
- Output DMAs ride the sync queue so they never contend with x prefetch.
- Block sizes <= 512 (PSUM bank), descending (512, ..., 384, 256): big
  early blocks hide the weight stream; the last down accumulator is split
  in halves to trim the end-of-kernel drain.
"""

import os
import sys

sys.path.insert(0, "/opt/trn_rl_repo")

import ml_dtypes
import numpy as np

# Problem dims (hardcoded per spec)
B, S, D, F, E, R = 2, 4096, 1024, 4096, 16, 16
TOPK = 4
N_TOK = B * S          # 8192
F2 = F // 2            # 2048 weight columns per core
DC = D // 128          # 8
FC = F2 // 128         # 16
WARM_N = 10            # PE p-state warmup matmuls
XB0_SPLIT = 1          # way-split of the first x block load
HAP_BUFS = FC + 2      # h tile pool depth
W1_GRP = 1             # fc slices per w1 copy
W2_GRP = 1             # fc slices per w2 copy
TAIL_SPLIT = True      # split last down accumulator to trim end drain
TAIL_OUTQ = False      # (sim: scalar-queue tail outs were slightly worse)

_programs = {}
LAST_RESULTS = None
LAST_PROGRAM = None


def _build_program(segments):
    """segments: tuple of (blocks, fc_lo, fc_hi, up_len). Each segment
    processes sum(blocks) gathered tokens against the fc range
    [fc_lo, fc_hi) of the weight tensors (the expert/F-quarter pairing
    described above); its up-projection streams only up_len columns."""
    import concourse.tile as tile
    from concourse import bacc, mybir

    BF16 = mybir.dt.bfloat16
    F32 = mybir.dt.float32
    AF = mybir.ActivationFunctionType

    n_pad = sum(sum(blocks) for blocks, _, _, _ in segments)
    ncol = n_pad // 128

    nc = bacc.Bacc("TRN2", target_bir_lowering=False, debug=False, num_devices=8)

    xTd = nc.dram_tensor("xT", [D, n_pad], BF16, kind="ExternalInput")
    w1d = nc.dram_tensor("w1", [128, FC * DC * 128], BF16, kind="ExternalInput")
    w2d = nc.dram_tensor("w2", [128, FC * D], BF16, kind="ExternalInput")
    wcd = nc.dram_tensor("wc", [128, ncol], F32, kind="ExternalInput")
    outd = nc.dram_tensor("out", [n_pad, D], BF16, kind="ExternalOutput")

    with tile.TileContext(nc) as tc:
        with (
            tc.tile_pool(name="singles", bufs=1) as singles,
            tc.tile_pool(name="xp", bufs=2) as xp,
            tc.tile_pool(name="hap", bufs=HAP_BUFS) as hap,
            tc.tile_pool(name="outp", bufs=3) as outp,
            tc.tile_pool(name="psH", bufs=3, space="PSUM") as psH,
            tc.tile_pool(name="psEO", bufs=5, space="PSUM") as psEO,
        ):
            # ---- resident weights ----
            w1 = singles.tile([128, FC, DC, 128], BF16)   # [p, fc, dc, q]
            w2 = singles.tile([128, FC, D], BF16)         # [p, fc, d]
            w_all = singles.tile([128, ncol], F32)

            xT_r = xTd.rearrange("(dc p) t -> p dc t", p=128)
            # w1/w2 arrive host-pre-swizzled in SBUF order: copies are
            # contiguous per-partition blits (128 descriptors, groupable)
            w1_r = w1d.rearrange("p (fc dc q) -> p fc dc q", fc=FC, dc=DC)
            w2_r = w2d.rearrange("p (fc d) -> p fc d", fc=FC)

            def load_block(t0, bs, nsplit=1):
                t = xp.tile([128, DC, bs], BF16, tag="xb")
                step = DC // nsplit
                for d0 in range(0, DC, step):
                    nc.scalar.dma_start(
                        t[:, d0:d0 + step, :], xT_r[:, d0:d0 + step, t0:t0 + bs]
                    )
                return t

            # flatten segments into a linear block schedule; each entry
            # carries bs_up <= bs: the exact token count the up-proj must
            # stream (down-proj stays 128-aligned; surplus h columns land
            # in zero-weight rows the host never reads)
            sched = []
            for blocks, fc_lo, fc_hi, up_len in segments:
                off = 0
                for bs in blocks:
                    bs_up = max(0, min(bs, up_len - off))
                    sched.append((bs, fc_lo, fc_hi, bs_up))
                    off += bs

            # need-ordered loads: segment A reads w1 fc0-7 from ~4us and
            # w2 fc0-7 from ~30us; the fc8-15 halves only at segment B
            xb = load_block(0, sched[0][0], nsplit=XB0_SPLIT)  # bs of block 0
            half = FC // 2
            for f0 in range(0, half, W1_GRP):
                f1 = min(f0 + W1_GRP, half)
                nc.sync.dma_start(w1[:, f0:f1, :, :], w1_r[:, f0:f1, :, :])
            nc.sync.dma_start(w_all[:], wcd[:, :])
            for f0 in range(0, half, W2_GRP):
                f1 = min(f0 + W2_GRP, half)
                nc.sync.dma_start(w2[:, f0:f1, :], w2_r[:, f0:f1, :])
            for f0 in range(half, FC, W1_GRP):
                f1 = min(f0 + W1_GRP, FC)
                nc.sync.dma_start(w1[:, f0:f1, :, :], w1_r[:, f0:f1, :, :])
            for f0 in range(half, FC, W2_GRP):
                f1 = min(f0 + W2_GRP, FC)
                nc.sync.dma_start(w2[:, f0:f1, :], w2_r[:, f0:f1, :])

            # PE p-state warmup (see module docstring)
            if WARM_N:
                warm = singles.tile([128, 512], BF16)
                nc.gpsimd.memset(warm[:], 0.0)
                ps_w = psEO.tile([128, 512], F32, tag="eo")
                for i in range(WARM_N):
                    nc.tensor.matmul(
                        ps_w[:], warm[:, :128], warm[:],
                        start=(i == 0), stop=(i == WARM_N - 1),
                    )

            t0 = 0
            for blk, (bs, fc_lo, fc_hi, bs_up) in enumerate(sched):
                # up projection: h[fc][:, t] = gelu(x @ W1c)[f, t]
                h_all = {}
                for fc in range(fc_lo, fc_hi):
                    ps_h = psH.tile([128, bs_up], F32, tag="psh")
                    for dc in range(DC):
                        nc.tensor.matmul(
                            ps_h[:], w1[:, fc, dc, :], xb[:, dc, :bs_up],
                            start=(dc == 0), stop=(dc == DC - 1),
                        )
                    h = hap.tile([128, bs], BF16, tag="h")
                    nc.scalar.activation(h[:, :bs_up], ps_h[:], AF.Gelu_apprx_tanh)
                    h_all[fc] = h

                # prefetch next block's x while the down passes run
                if blk + 1 < len(sched):
                    xb_next = load_block(t0 + bs, sched[blk + 1][0])
                else:
                    xb_next = None

                # down projection in two d-half passes, 128-token columns
                last_blk = blk == len(sched) - 1
                for dh in range(2):
                    for sub in range(bs // 128):
                        col = t0 // 128 + sub
                        r0 = t0 + sub * 128
                        # final accumulator split in halves: the first
                        # half's combine+DMA overlaps the second's matmuls,
                        # trimming the end-of-kernel drain
                        final = (TAIL_SPLIT and last_blk and dh == 1
                                 and sub == bs // 128 - 1)
                        for piece in ([(0, 256), (256, 512)] if final
                                      else [(0, 512)]):
                            p0, p1 = piece
                            pw = p1 - p0
                            eo = psEO.tile([128, pw], F32, tag="eo")
                            for fc in range(fc_lo, fc_hi):
                                nc.tensor.matmul(
                                    eo[:],
                                    h_all[fc][:, sub * 128:(sub + 1) * 128],
                                    w2[:, fc, dh * 512 + p0:dh * 512 + p1],
                                    start=(fc == fc_lo), stop=(fc == fc_hi - 1),
                                )
                            ob = outp.tile([128, pw], BF16, tag="ob")
                            nc.vector.tensor_scalar_mul(
                                ob[:], eo[:], scalar1=w_all[:, col:col + 1]
                            )
                            # last block's outputs ride the then-idle
                            # scalar queue (no xb prefetch left) so the
                            # final copy isn't stuck behind queued outs
                            oq = nc.scalar if (TAIL_OUTQ and last_blk) else nc.sync
                            oq.dma_start(
                                outd[r0:r0 + 128,
                                     dh * 512 + p0:dh * 512 + p1], ob[:]
                            )

                xb = xb_next
                t0 += bs

    nc.compile()
    return nc


def _get_program(segments):
    segments = tuple(segments)
    if segments not in _programs:
        _programs[segments] = _build_program(segments)
    return _programs[segments]


def _block_split(n_pad):
    """Split n_pad (multiple of 128) into blocks of <= 512 (PSUM bank),
    descending: big early blocks hide the weight-stream DMA (sim-verified
    faster than ascending; non-monotone orders broke PJRT execution)."""
    if n_pad <= 512:
        return (n_pad,)
    q, r = divmod(n_pad, 512)
    if r == 0:
        return (512,) * q
    if r == 128:
        # 384+256 instead of a 128 tail (keeps blocks >= 256)
        return (512,) * (q - 1) + (384, 256)
    return (512,) * q + (r,)


def _gate_weights(x2d, Wg):
    """Reference-faithful gate (same ops as the reference, jax on CPU so the
    fp32 softmax/top-4 selection matches bit-for-bit). Returns [N_TOK, 4]
    combine weights for experts 0-3."""
    try:
        import jax
        import jax.numpy as jnp
        cpu = jax.devices("cpu")[0]
        with jax.default_device(cpu):
            xf = jnp.asarray(x2d, jnp.float32)
            wg = jnp.asarray(Wg, jnp.float32)
            weights = jax.nn.softmax(xf @ wg, axis=-1)
            top_w, top_idx = jax.lax.top_k(weights, TOPK)
            top_w = top_w / jnp.sum(top_w, axis=-1, keepdims=True)
            cols = [jnp.sum(top_w * (top_idx == i), axis=-1) for i in range(TOPK)]
            return np.asarray(jnp.stack(cols, axis=-1), np.float32)
    except Exception:
        # numpy fallback (identical math, BLAS rounding may differ ~1e-7)
        logits = x2d.astype(np.float32) @ Wg.astype(np.float32)
        m = logits.max(axis=-1, keepdims=True)
        e = np.exp((logits - m).astype(np.float32), dtype=np.float32)
        p = (e / e.sum(axis=-1, keepdims=True).astype(np.float32)).astype(np.float32)
        idx = np.argsort(-p, axis=-1, kind="stable")[:, :TOPK]
        topw = np.take_along_axis(p, idx, axis=-1)
        topw = (topw / topw.sum(axis=-1, keepdims=True)).astype(np.float32)
        w = np.zeros((x2d.shape[0], TOPK), np.float32)
        for i in range(TOPK):
            w[:, i] = (topw * (idx == i)).sum(axis=-1)
        return w


def kernel(x, Wg, W1, A1, B1, W2, A2, B2):
    global LAST_RESULTS, LAST_PROGRAM
    from concourse.bass_utils import run_bass_kernel_spmd

    x = np.asarray(x, dtype=np.float32)
    x2d = x.reshape(N_TOK, D)
    w4 = _gate_weights(x2d, np.asarray(Wg, dtype=np.float32))

    # gather contributing tokens per expert (combine weight exactly 0 else)
    idxs = [np.nonzero(w4[:, e])[0] for e in range(TOPK)]
    counts = [len(ix) for ix in idxs]
    pads = [max(128, -(-c // 128) * 128) for c in counts]

    # 16 work units (expert, F-quarter), each sized pads[e]. Pair the 8
    # largest with the 8 smallest so every core gets an equal token budget
    # (expert imbalance otherwise pads every core to the largest expert).
    units = sorted(
        ((pads[e], e, q) for e in range(TOPK) for q in range(4)), reverse=True
    )
    big, small = units[:8], units[8:]
    nA, nB = big[0][0], small[0][0]
    upA = max(counts[e] for _, e, _ in big)
    upB = max(counts[e] for _, e, _ in small)
    segments = ((_block_split(nA), 0, FC // 2, upA),
                (_block_split(nB), FC // 2, FC, upB))
    n_pad = nA + nB
    ncol = n_pad // 128
    FQ = F // 4  # 1024 weight columns per quarter

    nc = _get_program(segments)
    LAST_PROGRAM = nc

    bf16 = ml_dtypes.bfloat16
    x2dT_b = x2d.T.astype(bf16)  # [D, N] in bf16
    folded = []
    for e in range(TOPK):
        # fold the rank-16 LoRA into the dense weights (exact identity)
        w1c = (np.asarray(W1[e], np.float64)
               + np.asarray(A1[e], np.float64) @ np.asarray(B1[e], np.float64))
        w2c = (np.asarray(W2[e], np.float64)
               + np.asarray(A2[e], np.float64) @ np.asarray(B2[e], np.float64))
        folded.append((w1c.astype(bf16), w2c.astype(bf16)))

    in_maps = []
    placements = []  # per core: ((eA, cA), (eB, cB)) for output assembly
    for core in range(8):
        (szA, eA, qA), (szB, eB, qB) = big[core], small[core]
        xg = np.zeros((D, n_pad), bf16)
        xg[:, :counts[eA]] = x2dT_b[:, idxs[eA]]
        xg[:, nA:nA + counts[eB]] = x2dT_b[:, idxs[eB]]
        wg = np.zeros(n_pad, np.float32)
        wg[:counts[eA]] = w4[idxs[eA], eA]
        wg[nA:nA + counts[eB]] = w4[idxs[eB], eB]
        wc = np.ascontiguousarray(wg.reshape(ncol, 128).T)
        w1A, w2A = folded[eA]
        w1B, w2B = folded[eB]
        w1 = np.hstack([w1A[:, qA * FQ:(qA + 1) * FQ],
                        w1B[:, qB * FQ:(qB + 1) * FQ]])
        w2 = np.vstack([w2A[qA * FQ:(qA + 1) * FQ, :],
                        w2B[qB * FQ:(qB + 1) * FQ, :]])
        # swizzle to the SBUF layouts so device copies are contiguous
        # per-partition blits: w1 [p, fc, dc, q], w2 [p, fc, d]
        w1s = w1.reshape(DC, 128, FC, 128).transpose(1, 2, 0, 3).reshape(128, -1)
        w2s = w2.reshape(FC, 128, D).transpose(1, 0, 2).reshape(128, -1)
        in_maps.append({
            "xT": xg,
            "w1": np.ascontiguousarray(w1s),
            "w2": np.ascontiguousarray(w2s),
            "wc": wc,
        })
        placements.append(((eA, counts[eA]), (eB, counts[eB])))

    trace = bool(os.environ.get("KERNEL_TRACE"))
    res = None
    last_exc = None
    for _attempt in range(3):
        try:
            res = run_bass_kernel_spmd(
                nc, in_maps, core_ids=list(range(8)), trace=trace
            )
            break
        except Exception as exc:  # transient NRT/profiling faults — retry
            last_exc = exc
            trace = False
    if res is None:
        raise last_exc
    LAST_RESULTS = res

    out = np.zeros((N_TOK, D), np.float64)
    for core in range(8):
        o = res.results[core]["out"]
        (eA, cA), (eB, cB) = placements[core]
        out[idxs[eA]] += o[:cA].astype(np.float64)
        out[idxs[eB]] += o[nA:nA + cB].astype(np.float64)
    return out.astype(np.float32).reshape(B, S, D)


# revision 45
# speedup vs baseline: 1.0151x; 1.0044x over previous
"""MoE layer (nn_MoELayer_4681514353281) Trainium2 Bass kernel.

Reference semantics: for slot i in range(4), expert i's FFN (W1 + A1@B1 LoRA,
gelu-tanh, W2 + A2@B2 LoRA) runs densely over ALL tokens; per-token combine
weight = renormalized top-4 softmax gate weight where top_idx == i (else 0).
Only experts 0-3 are ever used.

Token gathering: a token contributes to expert i only when i is in its top-4
(~1/4 of tokens per expert), so each core processes just the gathered
contributing tokens (~2.1k instead of all 8192) — an exact 4x FLOP cut, since
dropped (token, expert) pairs have combine weight exactly 0.

Sharding: 8 cores x 2 segments = 16 work units (expert, F-quarter). Each core
gets one unit from the 8 largest and one from the 8 smallest (pairing), so
expert token-count imbalance doesn't pad every core to the largest expert.
A core's weight input [D, 2048] holds the two quarter-slices of W1c/W2c
(W1c = W1 + A1@B1, W2c = W2 + A2@B2 — LoRA folded on host, exact identity);
its x input concatenates the two gathered token streams. Segment A uses
weight columns fc 0-7, segment B fc 8-15. Host scatter-adds the 16 partial
outputs (4 F-quarter partials per (token, expert) pair).

The gate's top-4 selection needs ~1e-6 logit precision to reproduce the fp32
reference's picks (near-ties flip otherwise), so the 8192x16 softmax/top-4
(0.13% of FLOPs) is computed on the host. FFN operands are bf16 (same
78.6 TF/s PE rate as float32r, half the DMA/SBUF, no small-moving-dim
penalty); accumulation is fp32 in PSUM. bf16 adds ~3e-3 relative error,
well inside the 2e-2 gate.

Schedule notes (TimelineSim-tuned):
- DMA loads are need-ordered: x block 0, w1 fc0-7, wc, w2 fc0-7, then the
  segment-B halves. HWDGE costs ~630ns per copy (128 descriptors) so
  startup-critical copies must not queue behind later ones.
- 10 warmup matmuls on a zeroed tile keep the PE busy until the first real
  operands land: an idle gap resets the PE p-state ramp (0.65->2.4GHz
  after 3us continuous), which would halve early matmul throughput.
- Output DMAs ride the sync queue so they never contend with x prefetch.
- Block sizes <= 512 (PSUM bank), descending (512, ..., 384, 256): big
  early blocks hide the weight stream; the last down accumulator is split
  in halves to trim the end-of-kernel drain.
"""

import os
import sys

sys.path.insert(0, "/opt/trn_rl_repo")

import ml_dtypes
import numpy as np

# Problem dims (hardcoded per spec)
B, S, D, F, E, R = 2, 4096, 1024, 4096, 16, 16
TOPK = 4
N_TOK = B * S          # 8192
F2 = F // 2            # 2048 weight columns per core
DC = D // 128          # 8
FC = F2 // 128         # 16
WARM_N = 8             # PE p-state warmup matmuls
XB0_SPLIT = 2          # way-split of the first x block load
HAP_BUFS = FC + 2      # h tile pool depth
W1_GRP = 1             # fc slices per w1 copy
W2_GRP = 1             # fc slices per w2 copy
TAIL_SPLIT = True      # split last down accumulator to trim end drain
TAIL_OUTQ = False      # (sim: scalar-queue tail outs were slightly worse)

_programs = {}
LAST_RESULTS = None
LAST_PROGRAM = None


def _build_program(segments):
    """segments: tuple of (blocks, fc_lo, fc_hi, up_len). Each segment
    processes sum(blocks) gathered tokens against the fc range
    [fc_lo, fc_hi) of the weight tensors (the expert/F-quarter pairing
    described above); its up-projection streams only up_len columns."""
    import concourse.tile as tile
    from concourse import bacc, mybir

    BF16 = mybir.dt.bfloat16
    F32 = mybir.dt.float32
    AF = mybir.ActivationFunctionType

    n_pad = sum(sum(blocks) for blocks, _, _, _ in segments)
    ncol = n_pad // 128

    nc = bacc.Bacc("TRN2", target_bir_lowering=False, debug=False, num_devices=8)

    xTd = nc.dram_tensor("xT", [D, n_pad], BF16, kind="ExternalInput")
    w1d = nc.dram_tensor("w1", [128, FC * DC * 128], BF16, kind="ExternalInput")
    w2d = nc.dram_tensor("w2", [128, FC * D], BF16, kind="ExternalInput")
    wcd = nc.dram_tensor("wc", [128, ncol], F32, kind="ExternalInput")
    outd = nc.dram_tensor("out", [n_pad, D], BF16, kind="ExternalOutput")

    with tile.TileContext(nc) as tc:
        with (
            tc.tile_pool(name="singles", bufs=1) as singles,
            tc.tile_pool(name="xp", bufs=2) as xp,
            tc.tile_pool(name="hap", bufs=HAP_BUFS) as hap,
            tc.tile_pool(name="outp", bufs=3) as outp,
            tc.tile_pool(name="psH", bufs=3, space="PSUM") as psH,
            tc.tile_pool(name="psEO", bufs=5, space="PSUM") as psEO,
        ):
            # ---- resident weights ----
            w1 = singles.tile([128, FC, DC, 128], BF16)   # [p, fc, dc, q]
            w2 = singles.tile([128, FC, D], BF16)         # [p, fc, d]
            w_all = singles.tile([128, ncol], F32)

            xT_r = xTd.rearrange("(dc p) t -> p dc t", p=128)
            # w1/w2 arrive host-pre-swizzled in SBUF order: copies are
            # contiguous per-partition blits (128 descriptors, groupable)
            w1_r = w1d.rearrange("p (fc dc q) -> p fc dc q", fc=FC, dc=DC)
            w2_r = w2d.rearrange("p (fc d) -> p fc d", fc=FC)

            def load_block(t0, bs, nsplit=1):
                t = xp.tile([128, DC, bs], BF16, tag="xb")
                step = DC // nsplit
                for d0 in range(0, DC, step):
                    nc.scalar.dma_start(
                        t[:, d0:d0 + step, :], xT_r[:, d0:d0 + step, t0:t0 + bs]
                    )
                return t

            # flatten segments into a linear block schedule; each entry
            # carries bs_up <= bs: the exact token count the up-proj must
            # stream (down-proj stays 128-aligned; surplus h columns land
            # in zero-weight rows the host never reads)
            sched = []
            for blocks, fc_lo, fc_hi, up_len in segments:
                off = 0
                for bs in blocks:
                    bs_up = max(0, min(bs, up_len - off))
                    sched.append((bs, fc_lo, fc_hi, bs_up))
                    off += bs

            # need-ordered loads: segment A reads w1 fc0-7 from ~4us and
            # w2 fc0-7 from ~30us; the fc8-15 halves only at segment B
            xb = load_block(0, sched[0][0], nsplit=XB0_SPLIT)  # bs of block 0
            half = FC // 2
            for f0 in range(0, half, W1_GRP):
                f1 = min(f0 + W1_GRP, half)
                nc.sync.dma_start(w1[:, f0:f1, :, :], w1_r[:, f0:f1, :, :])
            nc.sync.dma_start(w_all[:], wcd[:, :])
            for f0 in range(0, half, W2_GRP):
                f1 = min(f0 + W2_GRP, half)
                nc.sync.dma_start(w2[:, f0:f1, :], w2_r[:, f0:f1, :])
            for f0 in range(half, FC, W1_GRP):
                f1 = min(f0 + W1_GRP, FC)
                nc.sync.dma_start(w1[:, f0:f1, :, :], w1_r[:, f0:f1, :, :])
            for f0 in range(half, FC, W2_GRP):
                f1 = min(f0 + W2_GRP, FC)
                nc.sync.dma_start(w2[:, f0:f1, :], w2_r[:, f0:f1, :])

            # PE p-state warmup (see module docstring)
            if WARM_N:
                warm = singles.tile([128, 512], BF16)
                nc.gpsimd.memset(warm[:], 0.0)
                ps_w = psEO.tile([128, 512], F32, tag="eo")
                for i in range(WARM_N):
                    nc.tensor.matmul(
                        ps_w[:], warm[:, :128], warm[:],
                        start=(i == 0), stop=(i == WARM_N - 1),
                    )

            t0 = 0
            for blk, (bs, fc_lo, fc_hi, bs_up) in enumerate(sched):
                # up projection: h[fc][:, t] = gelu(x @ W1c)[f, t]
                h_all = {}
                for fc in range(fc_lo, fc_hi):
                    ps_h = psH.tile([128, bs_up], F32, tag="psh")
                    for dc in range(DC):
                        nc.tensor.matmul(
                            ps_h[:], w1[:, fc, dc, :], xb[:, dc, :bs_up],
                            start=(dc == 0), stop=(dc == DC - 1),
                        )
                    h = hap.tile([128, bs], BF16, tag="h")
                    nc.scalar.activation(h[:, :bs_up], ps_h[:], AF.Gelu_apprx_tanh)
                    h_all[fc] = h

                # prefetch next block's x while the down passes run
                if blk + 1 < len(sched):
                    xb_next = load_block(t0 + bs, sched[blk + 1][0])
                else:
                    xb_next = None

                # down projection in two d-half passes, 128-token columns
                last_blk = blk == len(sched) - 1
                for dh in range(2):
                    for sub in range(bs // 128):
                        col = t0 // 128 + sub
                        r0 = t0 + sub * 128
                        # final accumulator split in halves: the first
                        # half's combine+DMA overlaps the second's matmuls,
                        # trimming the end-of-kernel drain
                        final = (TAIL_SPLIT and last_blk and dh == 1
                                 and sub == bs // 128 - 1)
                        for piece in ([(0, 256), (256, 512)] if final
                                      else [(0, 512)]):
                            p0, p1 = piece
                            pw = p1 - p0
                            eo = psEO.tile([128, pw], F32, tag="eo")
                            for fc in range(fc_lo, fc_hi):
                                nc.tensor.matmul(
                                    eo[:],
                                    h_all[fc][:, sub * 128:(sub + 1) * 128],
                                    w2[:, fc, dh * 512 + p0:dh * 512 + p1],
                                    start=(fc == fc_lo), stop=(fc == fc_hi - 1),
                                )
                            ob = outp.tile([128, pw], BF16, tag="ob")
                            nc.vector.tensor_scalar_mul(
                                ob[:], eo[:], scalar1=w_all[:, col:col + 1]
                            )
                            # last block's outputs ride the then-idle
                            # scalar queue (no xb prefetch left) so the
                            # final copy isn't stuck behind queued outs
                            oq = nc.scalar if (TAIL_OUTQ and last_blk) else nc.sync
                            oq.dma_start(
                                outd[r0:r0 + 128,
                                     dh * 512 + p0:dh * 512 + p1], ob[:]
                            )

                xb = xb_next
                t0 += bs

    nc.compile()
    return nc


def _get_program(segments):
    segments = tuple(segments)
    if segments not in _programs:
        _programs[segments] = _build_program(segments)
    return _programs[segments]


def _block_split(n_pad):
    """Split n_pad (multiple of 128) into blocks of <= 512 (PSUM bank),
    descending: big early blocks hide the weight-stream DMA (sim-verified
    faster than ascending; non-monotone orders broke PJRT execution)."""
    if n_pad <= 512:
        return (n_pad,)
    q, r = divmod(n_pad, 512)
    if r == 0:
        return (512,) * q
    if r == 128:
        # 384+256 instead of a 128 tail (keeps blocks >= 256)
        return (512,) * (q - 1) + (384, 256)
    return (512,) * q + (r,)


def _gate_weights(x2d, Wg):
    """Reference-faithful gate (same ops as the reference, jax on CPU so the
    fp32 softmax/top-4 selection matches bit-for-bit). Returns [N_TOK, 4]
    combine weights for experts 0-3."""
    try:
        import jax
        import jax.numpy as jnp
        cpu = jax.devices("cpu")[0]
        with jax.default_device(cpu):
            xf = jnp.asarray(x2d, jnp.float32)
            wg = jnp.asarray(Wg, jnp.float32)
            weights = jax.nn.softmax(xf @ wg, axis=-1)
            top_w, top_idx = jax.lax.top_k(weights, TOPK)
            top_w = top_w / jnp.sum(top_w, axis=-1, keepdims=True)
            cols = [jnp.sum(top_w * (top_idx == i), axis=-1) for i in range(TOPK)]
            return np.asarray(jnp.stack(cols, axis=-1), np.float32)
    except Exception:
        # numpy fallback (identical math, BLAS rounding may differ ~1e-7)
        logits = x2d.astype(np.float32) @ Wg.astype(np.float32)
        m = logits.max(axis=-1, keepdims=True)
        e = np.exp((logits - m).astype(np.float32), dtype=np.float32)
        p = (e / e.sum(axis=-1, keepdims=True).astype(np.float32)).astype(np.float32)
        idx = np.argsort(-p, axis=-1, kind="stable")[:, :TOPK]
        topw = np.take_along_axis(p, idx, axis=-1)
        topw = (topw / topw.sum(axis=-1, keepdims=True)).astype(np.float32)
        w = np.zeros((x2d.shape[0], TOPK), np.float32)
        for i in range(TOPK):
            w[:, i] = (topw * (idx == i)).sum(axis=-1)
        return w


def kernel(x, Wg, W1, A1, B1, W2, A2, B2):
    global LAST_RESULTS, LAST_PROGRAM
    from concourse.bass_utils import run_bass_kernel_spmd

    x = np.asarray(x, dtype=np.float32)
    x2d = x.reshape(N_TOK, D)
    w4 = _gate_weights(x2d, np.asarray(Wg, dtype=np.float32))

    # gather contributing tokens per expert (combine weight exactly 0 else)
    idxs = [np.nonzero(w4[:, e])[0] for e in range(TOPK)]
    counts = [len(ix) for ix in idxs]
    pads = [max(128, -(-c // 128) * 128) for c in counts]

    # 16 work units (expert, F-quarter), each sized pads[e]. Pair the 8
    # largest with the 8 smallest so every core gets an equal token budget
    # (expert imbalance otherwise pads every core to the largest expert).
    units = sorted(
        ((pads[e], e, q) for e in range(TOPK) for q in range(4)), reverse=True
    )
    big, small = units[:8], units[8:]
    nA, nB = big[0][0], small[0][0]
    upA = max(counts[e] for _, e, _ in big)
    upB = max(counts[e] for _, e, _ in small)
    segments = ((_block_split(nA), 0, FC // 2, upA),
                (_block_split(nB), FC // 2, FC, upB))
    n_pad = nA + nB
    ncol = n_pad // 128
    FQ = F // 4  # 1024 weight columns per quarter

    nc = _get_program(segments)
    LAST_PROGRAM = nc

    bf16 = ml_dtypes.bfloat16
    x2dT_b = x2d.T.astype(bf16)  # [D, N] in bf16
    folded = []
    for e in range(TOPK):
        # fold the rank-16 LoRA into the dense weights (exact identity)
        w1c = (np.asarray(W1[e], np.float64)
               + np.asarray(A1[e], np.float64) @ np.asarray(B1[e], np.float64))
        w2c = (np.asarray(W2[e], np.float64)
               + np.asarray(A2[e], np.float64) @ np.asarray(B2[e], np.float64))
        folded.append((w1c.astype(bf16), w2c.astype(bf16)))

    in_maps = []
    placements = []  # per core: ((eA, cA), (eB, cB)) for output assembly
    for core in range(8):
        (szA, eA, qA), (szB, eB, qB) = big[core], small[core]
        xg = np.zeros((D, n_pad), bf16)
        xg[:, :counts[eA]] = x2dT_b[:, idxs[eA]]
        xg[:, nA:nA + counts[eB]] = x2dT_b[:, idxs[eB]]
        wg = np.zeros(n_pad, np.float32)
        wg[:counts[eA]] = w4[idxs[eA], eA]
        wg[nA:nA + counts[eB]] = w4[idxs[eB], eB]
        wc = np.ascontiguousarray(wg.reshape(ncol, 128).T)
        w1A, w2A = folded[eA]
        w1B, w2B = folded[eB]
        w1 = np.hstack([w1A[:, qA * FQ:(qA + 1) * FQ],
                        w1B[:, qB * FQ:(qB + 1) * FQ]])
        w2 = np.vstack([w2A[qA * FQ:(qA + 1) * FQ, :],
                        w2B[qB * FQ:(qB + 1) * FQ, :]])
        # swizzle to the SBUF layouts so device copies are contiguous
        # per-partition blits: w1 [p, fc, dc, q], w2 [p, fc, d]
        w1s = w1.reshape(DC, 128, FC, 128).transpose(1, 2, 0, 3).reshape(128, -1)
        w2s = w2.reshape(FC, 128, D).transpose(1, 0, 2).reshape(128, -1)
        in_maps.append({
            "xT": xg,
            "w1": np.ascontiguousarray(w1s),
            "w2": np.ascontiguousarray(w2s),
            "wc": wc,
        })
        placements.append(((eA, counts[eA]), (eB, counts[eB])))

    trace = bool(os.environ.get("KERNEL_TRACE"))
    res = None
    last_exc = None
    for _attempt in range(3):
        try:
            res = run_bass_kernel_spmd(
                nc, in_maps, core_ids=list(range(8)), trace=trace
            )
            break
        except Exception as exc:  # transient NRT/profiling faults — retry
            last_exc = exc
            trace = False
    if res is None:
        raise last_exc
    LAST_RESULTS = res

    out = np.zeros((N_TOK, D), np.float64)
    for core in range(8):
        o = res.results[core]["out"]
        (eA, cA), (eB, cB) = placements[core]
        out[idxs[eA]] += o[:cA].astype(np.float64)
        out[idxs[eB]] += o[nA:nA + cB].astype(np.float64)
    return out.astype(np.float32).reshape(B, S, D)
